# revision 17
# baseline (speedup 1.0000x reference)
"""Trainium2 Bass kernel for nn_CachePredictor (moe_routing).

Computation (see reference):
    x = relu(feature @ W_up.T + b_up)                      [B, 512]
    t_out = sigmoid(einsum('bf,bgf', x, W_table[tids]) + b_table[tids]) * tmask
    i_out = sigmoid(einsum('bf,bgf', x, W_index[iids]) + b_index[iids]) * imask
    out = stack([t_out, i_out])                            [2, B, 256]

Strategy: expert sharding (grouping samples by expert reads each expert
matrix exactly once, vs ~4 GB for per-sample gathers). Each of the 8 cores
owns 8 table + 16 index experts and processes only samples routed to its
experts; the host routes, remaps experts, pads, and does the trivial
elementwise tail (bias + sigmoid) during unscatter.

The kernel is HBM-stream-bound (~3.6 MB/core through the ~358 GB/s port),
and the measured window also carries ~9.4 us of fixed framework overhead
(Bass preamble + the walrus semaphore-reset epilogue), so the design
minimizes bytes and keeps the stream saturated end-to-end:

- Expert weights stored in HBM as fp8 e3m4 (scaled x32, clipped) and fed
  straight to the PE as the moving operand while x stays bf16. The x32 is
  compensated exactly by folding /32 into W_up/b_up.
- Experts are ranked by occupancy into BANDS of 8 (one expert per core per
  band), giving PER-BAND capacities instead of one global max: column
  padding drops ~25% vs a uniform capacity. Bands are paired big-with-
  small; a pair's segment is [expert-A rows | expert-B rows] with a
  COMPILE-TIME intra-pair boundary, so the device evacuates only the
  DIAGONAL blocks psum[0:wA, 0:256] / psum[wA:W, 256:512] - output bytes
  HALVE vs shipping both experts' logits for every sample, and the host
  tail needs no parity select. (Engine partition windows must start at 0
  or 64, so the B block is copied from partition 0/64 downward first and
  the A block then overwrites the garbage rows - costs nothing, copies
  are partition-parallel.)
- The device returns LOGITS in fp16; the host adds expert biases and
  applies sigmoid during unscatter.
- Outputs leave as FOUR interleaved blocks ([rows, n_pairs*256] fp16 with
  1-2 KB contiguous rows - short rows throttle the DMA engines to packet
  rate) on the two HWDGE rings, last blocks smallest.
- DMA channels (each HWDGE ring holds ONE in-flight DMA and pays ~1.5 us
  dead time between entries; SWDGE streams entries back-to-back but has
  ~3.5 us first-data latency and needs multi-KB rows):
    sync ring:   wu|b_up|features combined, wtB, wi0B, out_i0, out_i1b
    scalar ring: wtA, then wi1B / out_t / out_i1a dispatched mid-kernel
                 by ACT in evacuation gaps (the ring slot is long free)
    SWDGE:       wi0A, wi1A (4 KB rows; dispatched at t0 so the ~3.5 us
                 ucode latency hides in the fw window)
- ~4.3 us of dummy warmup matmuls ramp the HAM clock gate (PE starts at
  1.2 GHz, doubles after ~3.4 us busy) during the input-DMA wait.
- Stage-1 relu+bias split ACT/DVE; a dummy 1-element Relu pulls the ACT
  table load into the startup window.
- Index-role stage 2 runs TWO pairs concurrently in disjoint 64-column
  halves of the PE array (PSUM base-partition 0/64 -> tile_position
  (0,64)), doubling PE throughput where pair widths fit in 64.

Masked-off samples are never routed (reference zeroes them); the host
scatters computed rows back and leaves the rest zero.
"""

import ml_dtypes
import numpy as np

_N_CORES = 8
_F = 256        # feature dim
_HID = 512      # up-projection width
_G = 256        # buckets
_N_TABLES = 64
_N_INDEXES = 128
_TPC = _N_TABLES // _N_CORES    # table experts per core (8) = bands
_IPC = _N_INDEXES // _N_CORES   # index experts per core (16) = bands
_CPE = 8                        # experts per weight chunk (1 MiB fp8)
_WSCALE = 32.0                  # fp8 weight scale (folded into W_up/b_up)

_nc_cache = {}

# Set by a test harness to capture HW profiles; harmless when unused.
TRACE = False
LAST_RESULTS = None


def _pair_layout(caps):
    """Pair band p with band nb-1-p. Returns per-pair (bandA, bandB, wA, wB,
    segment column offset); segment order is pair 0, 1, ..."""
    nb = len(caps)
    pairs = []
    off = 0
    for p in range(nb // 2):
        a, b = p, nb - 1 - p
        wA, wB = int(caps[a]), int(caps[b])
        pairs.append((a, b, wA, wB, off))
        off += wA + wB
    return pairs, off  # off == total columns


def _blocks(pairsT, pairsI):
    """Output block plan: (name, role, [pair indices], rows). Small blocks
    shipped as soon as their pairs are evacuated; the last two go on
    different rings in parallel."""
    def rows(src, lps):
        return max(src[p][2] + src[p][3] for p in lps)

    return [
        ("ota", "t", [0, 1], rows(pairsT, [0, 1])),
        ("otb", "t", [2, 3], rows(pairsT, [2, 3])),
        ("oi0", "i", [0, 1, 2, 3], rows(pairsI, [0, 1, 2, 3])),
        ("oi1a", "i", [4, 5], rows(pairsI, [4, 5])),
        ("oi1b", "i", [6, 7], rows(pairsI, [6, 7])),
    ]


def _build(capsT, capsI):
    """Build + compile the SPMD program for per-band capacities."""
    from concourse import bacc
    import concourse.tile as tile
    import concourse.mybir as mybir

    F32 = mybir.dt.float32
    BF16 = mybir.dt.bfloat16
    FP16 = mybir.dt.float16
    F8E3 = mybir.dt.float8e3
    AF = mybir.ActivationFunctionType

    pairsT, NT = _pair_layout(capsT)
    pairsI, NI = _pair_layout(capsI)
    ICH = _IPC // _CPE   # index weight chunks (2)
    NA = NT + NI
    blocks = _blocks(pairsT, pairsI)

    nc = bacc.Bacc(
        "TRN2",
        target_bir_lowering=False,
        debug=False,
        enable_asserts=False,
        num_devices=_N_CORES,
    )
    # combined wu | features | b_up input (one ring DMA)
    fw = nc.dram_tensor("fw", [128, 1024 + 2 * NA + 4], BF16, kind="ExternalInput").ap()
    # host-packed, partition-major: [p, e_local*1024 + c*256 + g]
    wt = nc.dram_tensor("wt", [128, _CPE * 4 * _G], F8E3, kind="ExternalInput").ap()
    wi = nc.dram_tensor("wi", [ICH, 128, _CPE * 4 * _G], F8E3, kind="ExternalInput").ap()
    # outputs: interleaved pair blocks, only the routed expert's 256 logits
    odram = {
        name: nc.dram_tensor(name, [rows, len(lps) * _G], FP16, kind="ExternalOutput")
        .ap()
        .rearrange("r (q g) -> r q g", g=_G)
        for (name, role, lps, rows) in blocks
    }

    wtv = wt.rearrange("p (e c g) -> p e c g", e=_CPE, c=4)
    wiv = [wi[ch].rearrange("p (e c g) -> p e c g", e=_CPE, c=4) for ch in range(ICH)]
    h = _CPE // 2

    with tile.TileContext(nc) as tc:
        with (
            tc.tile_pool(name="persist", bufs=1) as persist,
            tc.tile_pool(name="ps1pool", bufs=3, space="PSUM") as ps1pool,
            tc.tile_pool(name="ps2pool", bufs=5, space="PSUM") as ps2pool,
        ):
            # --- input tiles ---
            fw_sb = persist.tile(
                [128, 1024 + 2 * NA + 4], BF16, name="fw_sb", tag="fw_sb"
            )
            wt_sb = persist.tile([128, _CPE, 4, _G], F8E3, name="wt_sb", tag="wt_sb")
            wi_sb = persist.tile(
                [128, ICH, _CPE, 4, _G], F8E3, name="wi_sb", tag="wi_sb"
            )
            # output block tiles (persist: each written once, shipped once)
            o_sb = {
                name: persist.tile(
                    [128, len(lps), _G], FP16, name=f"o_{name}", tag=f"o_{name}"
                )
                for (name, role, lps, rows) in blocks
            }

            # --- DMA channel plan (see module docstring) ---
            nc.sync.dma_start(out=fw_sb, in_=fw)                    # sync #1
            nc.scalar.dma_start(out=wt_sb[:, :h], in_=wtv[:, :h])   # scalar #1
            # SWDGE's ~4 us first-data latency naturally keeps these out of
            # fw's early HBM window; no gate needed.
            nc.gpsimd.dma_start(out=wi_sb[:, 0, :h], in_=wiv[0][:, :h])  # SWDGE
            nc.gpsimd.dma_start(out=wi_sb[:, 1, :h], in_=wiv[1][:, :h])  # SWDGE
            nc.sync.dma_start(out=wt_sb[:, h:], in_=wtv[:, h:])          # sync #2
            nc.sync.dma_start(out=wi_sb[:, 0, h:], in_=wiv[0][:, h:])    # sync #3
            # wi1B rides the scalar ring but must not contend with fw in
            # the early window, and its ACT-dispatch must not sit ahead of
            # the stage-1 relus in ACT's stream: an fp8 corner-copy (Pool,
            # waits on wtA's completion) writes into wi1B's dest region, so
            # the scheduler cannot hoist the dispatch.
            nc.gpsimd.tensor_copy(
                out=wi_sb[0:1, 1, h, 0, 0:16], in_=wt_sb[0:1, 0, 0, 0:16]
            )

            wu_v = lambda c, m: fw_sb[:, c * 512 + m * 128 : c * 512 + (m + 1) * 128]
            f_v = lambda c: fw_sb[:, 1024 + c * NA : 1024 + (c + 1) * NA]

            # per-partition bias scalars must be f32 APs: one tiny DVE copy
            buc_sb = persist.tile([128, 4], F32, name="buc_sb", tag="buc_sb")
            nc.vector.tensor_copy(
                out=buc_sb, in_=fw_sb[:, 1024 + 2 * NA : 1024 + 2 * NA + 4]
            )

            # pull the ACT relu table load into the startup window
            dummy = persist.tile([1, 16], F32, name="dummy", tag="dummy")
            nc.vector.memset(dummy, 0.0)
            nc.scalar.activation(out=dummy, in_=dummy, func=AF.Relu)

            # PE warmup during the fw-DMA wait: ramp the HAM clock gate
            warm = persist.tile([128, 512], BF16, name="warm", tag="warm")
            nc.vector.memset(warm, 0.0)
            for _ in range(9):
                psw = ps1pool.tile([128, 512], F32, name="ps1", tag="ps1")
                nc.tensor.matmul(psw, lhsT=warm[:, :128], rhs=warm, start=True, stop=True)

            # --- stage 1: xT[512, cols] = relu(W_upT.T @ featT + b_up) ---
            x_sb = {}
            off = {"t": 0, "i": NT}
            for role, NC in (("t", NT), ("i", NI)):
                x_sb[role] = [
                    persist.tile([128, NC], BF16, name=f"x_{role}{m}", tag=f"x_{role}{m}")
                    for m in range(4)
                ]
            # interleave m-chunk pairs so consecutive matmuls hit different
            # PSUM banks (same-bank accumulation passes serialize the PE)
            for role, NC in (("t", NT), ("i", NI)):
                for n0 in range(0, NC, 512):
                    nw = min(512, NC - n0)
                    for m0 in (0, 2):
                        ps1s = {
                            m: ps1pool.tile([128, 512], F32, name="ps1", tag="ps1")
                            for m in (m0, m0 + 1)
                        }
                        for c in range(2):
                            for m in (m0, m0 + 1):
                                nc.tensor.matmul(
                                    ps1s[m][:, :nw],
                                    lhsT=wu_v(c, m),
                                    rhs=f_v(c)[:, off[role] + n0 : off[role] + n0 + nw],
                                    start=(c == 0),
                                    stop=(c == 1),
                                )
                        for m in (m0, m0 + 1):
                            if m < 2 and role == "t":
                                nc.scalar.activation(
                                    out=x_sb[role][m][:, n0 : n0 + nw],
                                    in_=ps1s[m][:, :nw],
                                    func=AF.Relu,
                                    bias=buc_sb[:, m : m + 1],
                                )
                            else:
                                nc.vector.tensor_scalar(
                                    out=x_sb[role][m][:, n0 : n0 + nw],
                                    in0=ps1s[m][:, :nw],
                                    scalar1=buc_sb[:, m : m + 1],
                                    scalar2=0.0,
                                    op0=mybir.AluOpType.add,
                                    op1=mybir.AluOpType.max,
                                )

            # dispatch wi1B on the scalar ring now: the corner-copy WAW dep
            # keeps it after the relus in ACT's stream
            nc.scalar.dma_start(out=wi_sb[:, 1, h:], in_=wiv[1][:, h:])  # scalar #2

            # --- stage 2 ---
            # Per pair: 4 K-chunk matmuls (rhs fp8, N=512 spanning both
            # experts) accumulate logits in PSUM; evacuate the diagonal
            # blocks as fp16 into the pair's slot of its output block.
            # evacuation round-robins ACT / DVE (Pool cannot read PSUM) so
            # the ~240 ns fixed cost per copy splits across two engines
            evac_rr = [0]
            evac_engs = (
                lambda dst_ap, src_ap: nc.scalar.activation(
                    out=dst_ap, in_=src_ap, func=AF.Copy
                ),
                lambda dst_ap, src_ap: nc.vector.tensor_copy(
                    out=dst_ap, in_=src_ap
                ),
            )

            def do_pairs(role, plist, w_view):
                """plist: (local lp in chunk, wA, wB, x seg offset, psum
                base, block name, slot in block). Pairs in one plist share
                a psum tile (col tiling)."""
                xs = x_sb[role]
                ps2 = ps2pool.tile([128, 512], F32, name="ps2", tag="ps2")
                for c in range(4):
                    for (lp, wA, wB, soff, base, bname, slot) in plist:
                        W = wA + wB
                        nc.tensor.matmul(
                            ps2[base : base + W, :],
                            lhsT=xs[c][:, soff : soff + W],
                            rhs=w_view[:, 2 * lp : 2 * lp + 2, c, :],
                            start=(c == 0),
                            stop=(c == 3),
                        )
                for (lp, wA, wB, soff, base, bname, slot) in plist:
                    W = wA + wB
                    dst = o_sb[bname]
                    flo = 64 if wA >= 64 else 0
                    for (r0, r1, g0) in ((flo, W, _G), (0, wA, 0)):  # B, then A
                        evac_engs[evac_rr[0] % 2](
                            dst[r0:r1, slot, :],
                            ps2[base + r0 : base + r1, g0 : g0 + _G],
                        )
                        evac_rr[0] += 1

            # block membership lookup: (role, pair index) -> (name, slot)
            bmap = {}
            for (name, role, lps, rows) in blocks:
                for slot, lp in enumerate(lps):
                    bmap[(role, lp)] = (name, slot)

            def plist_for(role, pairs_abs, duo):
                pl = []
                src = pairsT if role == "t" else pairsI
                for k, p in enumerate(pairs_abs):
                    a, b, wA, wB, soff = src[p]
                    name, slot = bmap[(role, p)]
                    base = 64 * k if duo else 0
                    pl.append((p % 4, wA, wB, soff, base, name, slot))
                return pl

            def emit2(role, p0, p1):
                src = pairsT if role == "t" else pairsI
                w_view = wt_sb if role == "t" else wi_sb[:, p0 // 4]
                if src[p0][2] + src[p0][3] <= 64 and src[p1][2] + src[p1][3] <= 64:
                    do_pairs(role, plist_for(role, [p0, p1], True), w_view)
                else:
                    for p in (p0, p1):
                        do_pairs(role, plist_for(role, [p], False), w_view)

            def ship(name, ring):
                rows = next(r for (n, _, _, r) in blocks if n == name)
                ring.dma_start(out=odram[name][:, :, :], in_=o_sb[name][:rows])

            # emission order follows weight-arrival time; each output block
            # ships the moment its pairs are evacuated, and the last two
            # blocks leave on different rings in parallel
            emit2("t", 0, 1)
            ship("ota", nc.scalar)
            emit2("t", 2, 3)
            ship("otb", nc.scalar)
            emit2("i", 0, 1)
            emit2("i", 2, 3)
            ship("oi0", nc.sync)
            emit2("i", 4, 5)
            ship("oi1a", nc.scalar)
            emit2("i", 6, 7)
            ship("oi1b", nc.sync)

    nc.compile()
    return nc


def _get_nc(capsT, capsI):
    key = (tuple(capsT), tuple(capsI))
    if key not in _nc_cache:
        _nc_cache[key] = _build(capsT, capsI)
    return _nc_cache[key]


def _pack_weights(W):
    """[CPE, G, HID] f32 -> [128, CPE*4*G] partition-major fp8 chunk,
    scaled by _WSCALE and clipped to the e3m4 range."""
    A = W.reshape(_CPE, _G, 4, 128)               # [e, g, c, p]
    A = np.ascontiguousarray(A.transpose(3, 0, 2, 1))  # [p, e, c, g]
    A = np.clip(A * _WSCALE, -15.5, 15.5)
    return A.reshape(128, _CPE * 4 * _G).astype(ml_dtypes.float8_e3m4)


def _plan_role(ids, mask, n_experts):
    """Rank experts by occupancy into bands of _N_CORES (band j, core c ->
    expert order[j*8+c]); per-band capacity = band max count rounded to 4."""
    counts = np.bincount(ids[mask], minlength=n_experts)
    order = np.argsort(-counts, kind="stable")
    nb = n_experts // _N_CORES
    caps = tuple(
        max(4, int(-(-counts[order[j * _N_CORES]] // 4) * 4)) for j in range(nb)
    )
    return counts, order, caps


def _sigmoid(x):
    return 1.0 / (1.0 + np.exp(-x))


def kernel(
    feature,
    table_ids,
    index_ids,
    table_mask,
    index_mask,
    W_up,
    b_up,
    W_table,
    b_table,
    W_index,
    b_index,
):
    global LAST_RESULTS
    from concourse.bass_utils import run_bass_kernel_spmd

    feature = np.ascontiguousarray(np.asarray(feature), dtype=np.float32)
    table_ids = np.asarray(table_ids).astype(np.int64)
    index_ids = np.asarray(index_ids).astype(np.int64)
    table_mask = np.asarray(table_mask).astype(bool)
    index_mask = np.asarray(index_mask).astype(bool)
    W_up = np.asarray(W_up, dtype=np.float32)
    b_up = np.asarray(b_up, dtype=np.float32)
    W_table = np.asarray(W_table, dtype=np.float32)
    b_table = np.asarray(b_table, dtype=np.float32)
    W_index = np.asarray(W_index, dtype=np.float32)
    b_index = np.asarray(b_index, dtype=np.float32)

    B = feature.shape[0]

    cnt_t, ord_t, caps_t = _plan_role(table_ids, table_mask, _N_TABLES)
    cnt_i, ord_i, caps_i = _plan_role(index_ids, index_mask, _N_INDEXES)
    pairsT, NT = _pair_layout(caps_t)
    pairsI, NI = _pair_layout(caps_i)
    NA = NT + NI
    blocks = _blocks(pairsT, pairsI)

    nc = _get_nc(caps_t, caps_i)

    W_upT = (W_up.T / _WSCALE).astype(ml_dtypes.bfloat16)
    wu_part = np.ascontiguousarray(
        W_upT.reshape(2, 128, _HID).transpose(1, 0, 2).reshape(128, 1024)
    )
    buc = (b_up / _WSCALE).reshape(4, 128).T.astype(ml_dtypes.bfloat16)

    # per (role, core): expert ids in weight-chunk e-order [A0,B0,A1,B1,...]
    # and sample lists per (pair, half)
    def core_plan(ids, mask, order, pairs, core):
        eids, samples = [], []
        for (a, b, wA, wB, soff) in pairs:
            for band in (a, b):
                E = int(order[band * _N_CORES + core])
                eids.append(E)
                samples.append(np.flatnonzero((ids == E) & mask))
        return eids, samples

    in_maps = []
    plan_t, plan_i = [], []
    for c in range(_N_CORES):
        et, st = core_plan(table_ids, table_mask, ord_t, pairsT, c)
        ei, si = core_plan(index_ids, index_mask, ord_i, pairsI, c)
        plan_t.append((et, st))
        plan_i.append((ei, si))

        fa_c = np.zeros((_F, NA), ml_dtypes.bfloat16)
        for (role_pairs, samp, base) in ((pairsT, st, 0), (pairsI, si, NT)):
            for p, (a, b, wA, wB, soff) in enumerate(role_pairs):
                sA, sB = samp[2 * p], samp[2 * p + 1]
                if len(sA):
                    fa_c[:, base + soff : base + soff + len(sA)] = feature[sA].T
                if len(sB):
                    fa_c[:, base + soff + wA : base + soff + wA + len(sB)] = (
                        feature[sB].T
                    )
        fw_c = np.concatenate(
            [wu_part,
             fa_c.reshape(2, 128, NA).transpose(1, 0, 2).reshape(128, 2 * NA),
             buc],
            axis=1,
        )
        in_maps.append(
            {
                "fw": fw_c,
                "wt": _pack_weights(W_table[et]),
                "wi": np.stack(
                    [
                        _pack_weights(W_index[ei[:_CPE]]),
                        _pack_weights(W_index[ei[_CPE:]]),
                    ]
                ),
            }
        )

    res = run_bass_kernel_spmd(
        nc, in_maps, core_ids=list(range(_N_CORES)), trace=TRACE
    )
    LAST_RESULTS = res

    # block membership lookup: (role, pair index) -> (name, slot)
    bmap = {}
    for (name, role, lps, rows) in blocks:
        for slot, lp in enumerate(lps):
            bmap[(role, lp)] = (name, slot)

    # unscatter + host elementwise tail: out = sigmoid(logit + bias)
    out = np.zeros((2, B, _G), np.float32)
    for c in range(_N_CORES):
        r = {
            name: res.results[c][name]
            .astype(np.float32)
            .reshape(rows, len(lps), _G)
            for (name, role, lps, rows) in blocks
        }
        for (ridx, pairs, (eids, samp), bias_tbl) in (
            (0, pairsT, plan_t[c], b_table),
            (1, pairsI, plan_i[c], b_index),
        ):
            role = "t" if ridx == 0 else "i"
            for p, (a, b, wA, wB, soff) in enumerate(pairs):
                name, slot = bmap[(role, p)]
                for half, w0 in ((0, 0), (1, wA)):
                    s = samp[2 * p + half]
                    if len(s):
                        E = eids[2 * p + half]
                        logit = r[name][w0 : w0 + len(s), slot, :]
                        out[ridx, s, :] = _sigmoid(logit + bias_tbl[E])
    return out
